# revision 1
# baseline (speedup 1.0000x reference)
"""LoRALinear (paged multi-adapter LoRA + base linear) Trainium2 kernel.

Full-input contract: kernel(**inputs) takes the unsharded tensors and
returns the full [T, D_OUT] output.

Sharding: tokens are split contiguously across the 8 NeuronCores
(1024 tokens/core).  The base weight, bias and the (tiny) LoRA page
caches are preprocessed on host into per-core dense operands:

  out_c = x_c @ W^T + bias + ((x_c @ A_c^T) * mask_c) @ B_c

where A_c/B_c stack the rank-64 page blocks of every adapter that
appears in core c's token range (G blocks, G==1 for the standard
equal-length-sequence layout) and mask_c[r, t] folds the per-token
adapter one-hot, the per-adapter rank mask and the per-sequence scaling
into one multiplier.  The bias is folded into the LoRA-B matmul as one
extra contraction row (ones row in the masked activations, bias row in
B).  All matmuls run as float32r (full PE rate for free dim >= 256).

Device schedule (per core):
  - x^T stays resident in SBUF (16 MiB); W^T streams through a small
    tile pool; per-k DMAs are interleaved (x slab, A slab, n=0 W tile)
    so the PE starts ~3us in instead of waiting for the full x load.
  - LoRA-A accumulators occupy PSUM banks until the x load finishes, so
    the n=0 output block only runs 6 of its 8 token-row tiles inline;
    the remaining 2 run as a deferred pass at the end (re-streaming
    n=0's W tiles, +8 MiB DMA, fully overlapped).
"""

import os

import numpy as np

import concourse.bass as bass
import concourse.bacc as bacc
import concourse.mybir as mybir
import concourse.tile as tile
from concourse.bass_utils import run_bass_kernel_spmd

N_CORES = 8
T = 8192
D_IN = 4096
D_OUT = 4096
TPC = T // N_CORES  # tokens per core
MAX_RANK = 64
P = 128
NFREE = 512  # matmul moving free dim (fp32 max)

F32 = mybir.dt.float32
F32R = mybir.dt.float32r

# exec time of the last device run (ns), when KERNEL_TRACE=1
last_exec_time_ns = None
last_results = None


def _rblocks(r_aug):
    """Split r_aug LoRA contraction rows into <=128-row blocks."""
    out = []
    start = 0
    while start < r_aug:
        cnt = min(P, r_aug - start)
        out.append((start, cnt))
        start += cnt
    return out


def _build_program(r_aug, d_in=D_IN, d_out=D_OUT, tpc=TPC,
                   w_bufs=12, o_bufs=4, out_dma="sync"):
    """Build the per-core Bass program.

    r_aug = G*64 + 1 LoRA contraction rows (last row = ones/bias).
    """
    k_tiles = d_in // P
    m_tiles = tpc // P
    n_tiles = d_out // NFREE
    t_chunks = tpc // NFREE
    rbs = _rblocks(r_aug)
    n_lora_ps = t_chunks * len(rbs)  # live LoRA-A psum tiles during startup
    # m-tiles of the n=0 block that fit alongside the LoRA-A accumulators
    m_inline = max(0, min(m_tiles, 8 - n_lora_ps))
    defer = list(range(m_inline, m_tiles))  # deferred to a tail pass

    nc = bacc.Bacc("TRN2", target_bir_lowering=False, debug=False)

    xT = nc.dram_tensor("xT", [d_in, tpc], F32R, kind="ExternalInput").ap()
    wT = nc.dram_tensor("wT", [d_in, d_out], F32R, kind="ExternalInput").ap()
    aT = nc.dram_tensor("aT", [d_in, r_aug], F32R, kind="ExternalInput").ap()
    bS = nc.dram_tensor("bS", [r_aug, d_out], F32R, kind="ExternalInput").ap()
    mS = nc.dram_tensor("mS", [r_aug, tpc], F32, kind="ExternalInput").ap()
    out = nc.dram_tensor("out", [tpc, d_out], F32, kind="ExternalOutput").ap()

    with tile.TileContext(nc) as tc:
        with (
            tc.tile_pool(name="xpool", bufs=k_tiles) as xpool,
            tc.tile_pool(name="cpool", bufs=1) as cpool,
            tc.tile_pool(name="wpool", bufs=w_bufs) as wpool,
            tc.tile_pool(name="opool", bufs=o_bufs) as opool,
            tc.tile_pool(name="psum", bufs=8, space="PSUM") as psum,
        ):
            # small resident inputs first (cheap DMAs, needed mid-flight)
            bss = {}
            mss = {}
            xam = {}
            for bi, (rs, rc) in enumerate(rbs):
                b_t = cpool.tile([rc, d_out], F32R, tag=f"bss{bi}",
                                 name=f"bss_{bi}")
                nc.sync.dma_start(b_t, bS[rs:rs + rc, :])
                bss[bi] = b_t
                m_t = cpool.tile([rc, tpc], F32, tag=f"mss{bi}",
                                 name=f"mss_{bi}")
                nc.sync.dma_start(m_t, mS[rs:rs + rc, :])
                mss[bi] = m_t
                xam[bi] = cpool.tile([rc, tpc], F32R, tag=f"xam{bi}",
                                     name=f"xam_{bi}")

            # ones/bias row lives in the last block's last row: copy it from
            # the mask now (also absorbs the mss DMA wait ahead of the muls).
            bl, (bl_rs, bl_rc) = len(rbs) - 1, rbs[-1]
            nc.vector.tensor_copy(xam[bl][bl_rc - 1:bl_rc, :],
                                  mss[bl][bl_rc - 1:bl_rc, :])

            # per-k interleaved loads: x slab, A slab, n=0 W tile.
            n0sl = slice(0, NFREE)
            xs = []
            ats = []
            wt0 = []
            for k in range(k_tiles):
                xt = xpool.tile([P, tpc], F32R, tag="xs", name=f"xs_{k}")
                nc.sync.dma_start(xt, xT[k * P:(k + 1) * P, :])
                xs.append(xt)
                at = cpool.tile([P, r_aug], F32R, tag="ats", bufs=k_tiles,
                                name=f"ats_{k}")
                nc.sync.dma_start(at, aT[k * P:(k + 1) * P, :])
                ats.append(at)
                wt = wpool.tile([P, NFREE], F32R, tag="wt", name=f"wt0_{k}")
                nc.sync.dma_start(wt, wT[k * P:(k + 1) * P, n0sl])
                wt0.append(wt)

            # LoRA-A accumulators: xamT[r, t] = sum_d A[r, d] x[t, d]
            lora_ps = {}
            for c in range(t_chunks):
                for bi, (rs, rc) in enumerate(rbs):
                    lora_ps[(c, bi)] = psum.tile([rc, NFREE], F32, tag="ps",
                                                 name=f"ps_lora_{c}_{bi}")
            # n=0 inline psum tiles
            psts0 = [psum.tile([P, NFREE], F32, tag="ps", name=f"pst_0_{i}")
                     for i in range(m_inline)]

            def copy_out(m, n, pst, idx):
                ot = opool.tile([P, NFREE], F32, tag="ot", name=f"ot_{n}_{m}")
                nc.vector.tensor_copy(ot, pst)
                # out_dma="scalar" rides the scalar engine's HWDGE queue so
                # stores don't sit behind the weight stream on the sync queue.
                getattr(nc, out_dma).dma_start(
                    out[m * P:(m + 1) * P, n * NFREE:(n + 1) * NFREE], ot)

            def lora_b(pst, m, nsl, stop):
                """Accumulate lora+bias rows into a base psum tile."""
                for bi, (rs, rc) in enumerate(rbs):
                    nc.tensor.matmul(
                        pst,
                        lhsT=xam[bi][:, m * P:(m + 1) * P],
                        rhs=bss[bi][:, nsl],
                        start=False,
                        stop=(stop and bi == len(rbs) - 1),
                    )

            # startup phase: per k, LoRA-A MMs + n=0 inline MMs
            for k in range(k_tiles):
                for c in range(t_chunks):
                    tsl = slice(c * NFREE, (c + 1) * NFREE)
                    for bi, (rs, rc) in enumerate(rbs):
                        nc.tensor.matmul(
                            lora_ps[(c, bi)],
                            lhsT=ats[k][:, rs:rs + rc],
                            rhs=xs[k][:, tsl],
                            start=(k == 0),
                            stop=(k == k_tiles - 1),
                        )
                for m in range(m_inline):
                    nc.tensor.matmul(
                        psts0[m],
                        lhsT=xs[k][:, m * P:(m + 1) * P],
                        rhs=wt0[k],
                        start=(k == 0),
                        stop=False,
                    )

            # masks: xam = lora_ps * mS (releases the LoRA psum tiles)
            for c in range(t_chunks):
                tsl = slice(c * NFREE, (c + 1) * NFREE)
                for bi, (rs, rc) in enumerate(rbs):
                    # last row of the last block is the ones row, keep it
                    rows = rc - 1 if bi == len(rbs) - 1 else rc
                    if rows:
                        nc.vector.tensor_mul(xam[bi][0:rows, tsl],
                                             lora_ps[(c, bi)][0:rows, :],
                                             mss[bi][0:rows, tsl])

            # finish n=0 inline m-tiles: lora rows + copy out
            for i, pst in enumerate(psts0):
                lora_b(pst, i, n0sl, stop=True)
                copy_out(i, 0, pst, i)

            # steady state: n = 1..n_tiles-1
            for n in range(1, n_tiles):
                nsl = slice(n * NFREE, (n + 1) * NFREE)
                psts = [psum.tile([P, NFREE], F32, tag="ps",
                                  name=f"pst_{n}_{i}") for i in range(m_tiles)]
                for k in range(k_tiles):
                    wt = wpool.tile([P, NFREE], F32R, tag="wt",
                                    name=f"wt_{n}_{k}")
                    nc.sync.dma_start(wt, wT[k * P:(k + 1) * P, nsl])
                    for m in range(m_tiles):
                        nc.tensor.matmul(
                            psts[m],
                            lhsT=xs[k][:, m * P:(m + 1) * P],
                            rhs=wt,
                            start=(k == 0),
                            stop=False,
                        )
                for m in range(m_tiles):
                    lora_b(psts[m], m, nsl, stop=True)
                    copy_out(m, n, psts[m], m)


            # deferred tail pass: n=0, m-tiles that were displaced by the
            # LoRA-A accumulators during startup (re-streams n=0 W tiles)
            if defer:
                pstd = [psum.tile([P, NFREE], F32, tag="ps",
                                  name=f"pstd_{i}") for i in defer]
                for k in range(k_tiles):
                    wt = wpool.tile([P, NFREE], F32R, tag="wt",
                                    name=f"wtd_{k}")
                    nc.sync.dma_start(wt, wT[k * P:(k + 1) * P, n0sl])
                    for j, m in enumerate(defer):
                        nc.tensor.matmul(
                            pstd[j],
                            lhsT=xs[k][:, m * P:(m + 1) * P],
                            rhs=wt,
                            start=(k == 0),
                            stop=False,
                        )
                for j, m in enumerate(defer):
                    lora_b(pstd[j], m, n0sl, stop=True)
                    copy_out(m, 0, pstd[j], j)

    nc.compile()
    return nc


def _prep_core_inputs(x, weight_t, bias, a_cache, b_cache, tok_adapter,
                      tok_scale, rank_page_table, ranks, core, g_max):
    """Host-side shard prep for one core."""
    d_in = x.shape[1]
    d_out = b_cache.shape[1]
    r = g_max * MAX_RANK
    sl = slice(core * TPC, (core + 1) * TPC)
    adapters = tok_adapter[sl]
    scales = tok_scale[sl]
    uniq = np.unique(adapters)

    aT = np.zeros((d_in, r + 1), np.float32)
    bS = np.zeros((r + 1, d_out), np.float32)
    mS = np.zeros((r + 1, TPC), np.float32)
    for g, a in enumerate(uniq):
        pages = rank_page_table[a]  # [64] page ids
        aT[:, g * MAX_RANK:(g + 1) * MAX_RANK] = a_cache[pages].T
        bS[g * MAX_RANK:(g + 1) * MAX_RANK, :] = b_cache[pages]
        slot_active = (np.arange(MAX_RANK) < ranks[a])[:, None]  # [64, 1]
        tok_active = (adapters == a)[None, :]  # [1, TPC]
        mS[g * MAX_RANK:(g + 1) * MAX_RANK, :] = (
            slot_active & tok_active) * scales[None, :]
    bS[r, :] = bias
    mS[r, :] = 1.0
    xT = np.ascontiguousarray(x[sl].T)
    return {"xT": xT, "wT": weight_t, "aT": np.ascontiguousarray(aT),
            "bS": bS, "mS": mS}


def kernel(x, weight, bias, a_cache, b_cache, b_start_loc, b_adapter_ids,
           b_scaling, rank_page_table, ranks):
    global last_exec_time_ns, last_results
    x = np.asarray(x, np.float32)
    weight = np.asarray(weight, np.float32)
    bias = np.asarray(bias, np.float32)
    a_cache = np.asarray(a_cache, np.float32)
    b_cache = np.asarray(b_cache, np.float32)
    b_start_loc = np.asarray(b_start_loc)
    b_adapter_ids = np.asarray(b_adapter_ids)
    b_scaling = np.asarray(b_scaling, np.float32)
    rank_page_table = np.asarray(rank_page_table)
    ranks = np.asarray(ranks)

    t = x.shape[0]
    seg = np.searchsorted(b_start_loc, np.arange(t, dtype=b_start_loc.dtype),
                          side="right") - 1
    tok_adapter = b_adapter_ids[seg]
    tok_scale = b_scaling[seg]

    g_max = max(
        len(np.unique(tok_adapter[c * TPC:(c + 1) * TPC]))
        for c in range(N_CORES)
    )
    r_aug = g_max * MAX_RANK + 1

    weight_t = np.ascontiguousarray(weight.T)
    in_maps = [
        _prep_core_inputs(x, weight_t, bias, a_cache, b_cache, tok_adapter,
                          tok_scale, rank_page_table, ranks, c, g_max)
        for c in range(N_CORES)
    ]

    nc = _build_program(r_aug)
    trace = os.environ.get("KERNEL_TRACE", "0") == "1"
    repeat = int(os.environ.get("KERNEL_REPEAT", "1"))
    times = []
    for _ in range(repeat):
        res = run_bass_kernel_spmd(nc, in_maps, core_ids=list(range(N_CORES)),
                                   trace=trace)
        times.append(res.exec_time_ns)
    last_exec_time_ns = (min(t for t in times if t is not None)
                         if any(t is not None for t in times) else None)
    last_results = res
    if repeat > 1:
        print("exec times:", times)
    return np.concatenate([res.results[c]["out"] for c in range(N_CORES)],
                          axis=0).astype(np.float32)



# revision 5
# speedup vs baseline: 1.0373x; 1.0373x over previous
"""LoRALinear (paged multi-adapter LoRA + base linear) Trainium2 kernel.

Full-input contract: kernel(**inputs) takes the unsharded tensors and
returns the full [T, D_OUT] output.

Sharding: tokens are split contiguously across the 8 NeuronCores
(1024 tokens/core).  The base weight, bias and the (tiny) LoRA page
caches are preprocessed on host into per-core dense operands:

  out_c = x_c @ W^T + bias + ((x_c @ A_c^T) * mask_c) @ B_c

All matmul operands are fp16 (rel err ~3e-4 vs the 2e-2 gate), which
runs the PE at the same 1 row/cycle as fp32r but halves weight-load
time and all DMA traffic.  PSUM accumulation stays fp32.  The bias is
folded into the LoRA-B matmul as one extra contraction row.

Device schedule (per core):
  - x^T (+ the 64 LoRA-A rows, fused into the same per-k slab DMA)
    stays resident in SBUF (~9 MiB fp16).
  - W is host-packed so each DMA is a [128, 2048] fp16 "quad" (4 KiB
    per partition row) holding 4 k-tiles of one n-block.
  - n=0's 8 quads stay resident, so the two m-tiles displaced by the
    LoRA-A PSUM accumulators during startup run right after the mask
    phase with zero extra DMA (the baseline re-streamed 8 MiB at the
    very end).
  - Output staging DMAs ride the scalar engine's queue and are split
    across DGE queues (the final tile's 256 KiB on one ~19 GB/s queue
    was a 12 us tail in the baseline).
"""

import os

import numpy as np

import concourse.bass as bass
import concourse.bacc as bacc
import concourse.mybir as mybir
import concourse.tile as tile
from concourse.bass_utils import run_bass_kernel_spmd

N_CORES = 8
T = 8192
D_IN = 4096
D_OUT = 4096
TPC = T // N_CORES  # tokens per core
MAX_RANK = 64
P = 128
NFREE = 512  # matmul moving free dim (psum bank)
KT = D_IN // P          # 32 k tiles
MT = TPC // P           # 8 token (m) tiles
NT = D_OUT // NFREE     # 8 output (n) blocks
QK = 4                  # k tiles per packed W quad
NQ = KT // QK           # 8 quads per n block
XAW = TPC + MAX_RANK    # fused x+A slab width (1088)

F32 = mybir.dt.float32
F16 = mybir.dt.float16

# exec time of the last device run (ns), when KERNEL_TRACE=1
last_exec_time_ns = None
last_results = None


def _build_program(r_aug=MAX_RANK + 1, d_out=D_OUT, tpc=TPC):
    """Build the per-core Bass program (G=1: 64 LoRA rows + bias row)."""
    t_chunks = tpc // NFREE  # 2
    m_inline = MT - t_chunks  # 6 m-tiles of n=0 run during startup
    defer = list(range(m_inline, MT))

    nc = bacc.Bacc("TRN2", target_bir_lowering=False, debug=False)

    xaT = nc.dram_tensor("xaT", [D_IN, XAW], F16, kind="ExternalInput").ap()
    wP = nc.dram_tensor("wP", [NQ * P, NT * QK * NFREE], F16,
                        kind="ExternalInput").ap()
    bS = nc.dram_tensor("bS", [r_aug, d_out], F16, kind="ExternalInput").ap()
    mS = nc.dram_tensor("mS", [r_aug, tpc], F32, kind="ExternalInput").ap()
    out = nc.dram_tensor("out", [tpc, d_out], F32, kind="ExternalOutput").ap()

    QW = QK * NFREE  # 2048 cols per quad

    with tile.TileContext(nc) as tc:
        with (
            tc.tile_pool(name="xapool", bufs=KT) as xapool,
            tc.tile_pool(name="w0pool", bufs=NQ) as w0pool,
            tc.tile_pool(name="cpool", bufs=1) as cpool,
            tc.tile_pool(name="wpool", bufs=4) as wpool,
            tc.tile_pool(name="opool", bufs=6) as opool,
            tc.tile_pool(name="psum", bufs=8, space="PSUM") as psum,
        ):
            # ---- DMA issue order: first k slab + first W quad lead ----
            xs = []
            w0 = []

            def xa_dma(k):
                t = xapool.tile([P, XAW], F16, tag="xa", name=f"xa_{k}")
                nc.sync.dma_start(t, xaT[k * P:(k + 1) * P, :])
                xs.append(t)

            def w0_dma(j):
                t = w0pool.tile([P, QW], F16, tag="w0", name=f"w0_{j}")
                nc.sync.dma_start(t, wP[j * P:(j + 1) * P, 0:QW])
                w0.append(t)

            xa_dma(0)
            w0_dma(0)
            bss = cpool.tile([r_aug, d_out], F16, tag="bss", name="bss")
            nc.sync.dma_start(bss, bS)
            mss = cpool.tile([r_aug, tpc], F32, tag="mss", name="mss")
            nc.sync.dma_start(mss, mS)
            xam = cpool.tile([r_aug, tpc], F16, tag="xam", name="xam")
            w0_dma(1)
            for k in range(1, KT):
                xa_dma(k)
                j = k // QK + 1  # quad prefetch ~1 quad ahead of use
                if k % QK == 0 and j < NQ:
                    w0_dma(j)

            # ones row for the bias contraction (mss row 64 is all 1.0)
            nc.vector.tensor_copy(xam[MAX_RANK:r_aug, :],
                                  mss[MAX_RANK:r_aug, :])

            # ---- PE startup: per k, LoRA-A + 6 inline n=0 m-tiles ----
            lora_ps = [psum.tile([MAX_RANK, NFREE], F32, tag="ps",
                                 name=f"ps_lora_{c}") for c in range(t_chunks)]
            psts0 = [psum.tile([P, NFREE], F32, tag="ps", name=f"pst0_{m}")
                     for m in range(m_inline)]
            for k in range(KT):
                wq = w0[k // QK]
                qs = (k % QK) * NFREE
                for c in range(t_chunks):
                    nc.tensor.matmul(
                        lora_ps[c],
                        lhsT=xs[k][:, tpc:tpc + MAX_RANK],
                        rhs=xs[k][:, c * NFREE:(c + 1) * NFREE],
                        start=(k == 0),
                        stop=(k == KT - 1),
                    )
                for m in range(m_inline):
                    nc.tensor.matmul(
                        psts0[m],
                        lhsT=xs[k][:, m * P:(m + 1) * P],
                        rhs=wq[:, qs:qs + NFREE],
                        start=(k == 0),
                        stop=False,
                    )

            # ---- masks: xam = lora_ps * mS (frees the two LoRA banks) ----
            for c in range(t_chunks):
                tsl = slice(c * NFREE, (c + 1) * NFREE)
                nc.vector.tensor_mul(xam[0:MAX_RANK, tsl],
                                     lora_ps[c],
                                     mss[0:MAX_RANK, tsl])

            def lora_b(pst, m, nsl):
                nc.tensor.matmul(
                    pst,
                    lhsT=xam[:, m * P:(m + 1) * P],
                    rhs=bss[:, nsl],
                    start=False,
                    stop=True,
                )

            def copy_out(m, n, pst, last=False):
                ot = opool.tile([P, NFREE], F32, tag="ot", name=f"ot_{n}_{m}")
                nc.vector.tensor_copy(ot, pst)
                # scalar engine's DGE queue, split so rows land on several
                # queues (a single 256 KiB store drains at ~19 GB/s)
                parts = 4 if last else 2
                rows = P // parts
                for i in range(parts):
                    rs = slice(i * rows, (i + 1) * rows)
                    nc.scalar.dma_start(
                        out[m * P + i * rows:m * P + (i + 1) * rows,
                            n * NFREE:(n + 1) * NFREE], ot[rs, :])

            # ---- n=0 completion: lora rows for inline tiles, then the two
            # deferred m-tiles straight from the resident quads ----
            n0sl = slice(0, NFREE)
            for m in range(m_inline):
                lora_b(psts0[m], m, n0sl)
                copy_out(m, 0, psts0[m])
            pstd = [psum.tile([P, NFREE], F32, tag="ps", name=f"pstd_{m}")
                    for m in defer]
            for k in range(KT):
                wq = w0[k // QK]
                qs = (k % QK) * NFREE
                for i, m in enumerate(defer):
                    nc.tensor.matmul(
                        pstd[i],
                        lhsT=xs[k][:, m * P:(m + 1) * P],
                        rhs=wq[:, qs:qs + NFREE],
                        start=(k == 0),
                        stop=False,
                    )
            for i, m in enumerate(defer):
                lora_b(pstd[i], m, n0sl)
                copy_out(m, 0, pstd[i])

            # ---- steady state: n = 1..7 ----
            for n in range(1, NT):
                nsl = slice(n * NFREE, (n + 1) * NFREE)
                psts = [psum.tile([P, NFREE], F32, tag="ps",
                                  name=f"pst_{n}_{m}") for m in range(MT)]
                for j in range(NQ):
                    wq = wpool.tile([P, QW], F16, tag="wt", name=f"wt_{n}_{j}")
                    nc.sync.dma_start(
                        wq, wP[j * P:(j + 1) * P, n * QW:(n + 1) * QW])
                    for q in range(QK):
                        k = j * QK + q
                        for m in range(MT):
                            nc.tensor.matmul(
                                psts[m],
                                lhsT=xs[k][:, m * P:(m + 1) * P],
                                rhs=wq[:, q * NFREE:(q + 1) * NFREE],
                                start=(k == 0),
                                stop=False,
                            )
                for m in range(MT):
                    lora_b(psts[m], m, nsl)
                    copy_out(m, n, psts[m], last=(n == NT - 1))

    nc.compile()
    return nc


def _prep_core_inputs(x16, weight_p, bias, a_cache, b_cache, tok_adapter,
                      tok_scale, rank_page_table, ranks, core):
    """Host-side shard prep for one core (single adapter per core)."""
    d_out = b_cache.shape[1]
    r_aug = MAX_RANK + 1
    sl = slice(core * TPC, (core + 1) * TPC)
    adapters = tok_adapter[sl]
    scales = tok_scale[sl]
    a = int(adapters[0])
    pages = rank_page_table[a]  # [64] page ids

    xaT = np.empty((D_IN, XAW), np.float16)
    xaT[:, :TPC] = x16[sl].T
    xaT[:, TPC:] = a_cache[pages].T.astype(np.float16)

    bS = np.empty((r_aug, d_out), np.float16)
    bS[:MAX_RANK] = b_cache[pages].astype(np.float16)
    bS[MAX_RANK] = bias.astype(np.float16)

    mS = np.empty((r_aug, TPC), np.float32)
    slot_active = (np.arange(MAX_RANK) < ranks[a])[:, None]  # [64, 1]
    tok_active = (adapters == a)[None, :]  # [1, TPC]
    mS[:MAX_RANK] = (slot_active & tok_active) * scales[None, :]
    mS[MAX_RANK] = 1.0
    return {"xaT": xaT, "wP": weight_p, "bS": bS, "mS": mS}


def kernel(x, weight, bias, a_cache, b_cache, b_start_loc, b_adapter_ids,
           b_scaling, rank_page_table, ranks):
    global last_exec_time_ns, last_results
    x = np.asarray(x, np.float32)
    weight = np.asarray(weight, np.float32)
    bias = np.asarray(bias, np.float32)
    a_cache = np.asarray(a_cache, np.float32)
    b_cache = np.asarray(b_cache, np.float32)
    b_start_loc = np.asarray(b_start_loc)
    b_adapter_ids = np.asarray(b_adapter_ids)
    b_scaling = np.asarray(b_scaling, np.float32)
    rank_page_table = np.asarray(rank_page_table)
    ranks = np.asarray(ranks)

    t = x.shape[0]
    seg = np.searchsorted(b_start_loc, np.arange(t, dtype=b_start_loc.dtype),
                          side="right") - 1
    tok_adapter = b_adapter_ids[seg]
    tok_scale = b_scaling[seg]
    # this schedule assumes one adapter per 1024-token core slice (the
    # spec's equal-length-sequence layout guarantees it)
    assert all(
        len(np.unique(tok_adapter[c * TPC:(c + 1) * TPC])) == 1
        for c in range(N_CORES)
    )

    x16 = x.astype(np.float16)
    # pack W^T into [NQ*128, NT*2048]: row j*128+p, col n*2048 + q*512 + c
    # holds W^T[(4j+q)*128 + p, n*512 + c]
    wt = np.ascontiguousarray(weight.T.astype(np.float16))
    weight_p = np.ascontiguousarray(
        wt.reshape(NQ, QK, P, NT, NFREE)
          .transpose(0, 2, 3, 1, 4)
          .reshape(NQ * P, NT * QK * NFREE))

    in_maps = [
        _prep_core_inputs(x16, weight_p, bias, a_cache, b_cache, tok_adapter,
                          tok_scale, rank_page_table, ranks, c)
        for c in range(N_CORES)
    ]

    nc = _build_program()
    trace = os.environ.get("KERNEL_TRACE", "0") == "1"
    repeat = int(os.environ.get("KERNEL_REPEAT", "1"))
    times = []
    for _ in range(repeat):
        res = run_bass_kernel_spmd(nc, in_maps, core_ids=list(range(N_CORES)),
                                   trace=trace)
        times.append(res.exec_time_ns)
    last_exec_time_ns = (min(t for t in times if t is not None)
                         if any(t is not None for t in times) else None)
    last_results = res
    if repeat > 1:
        print("exec times:", times)
    return np.concatenate([res.results[c]["out"] for c in range(N_CORES)],
                          axis=0).astype(np.float32)
